# revision 2
# baseline (speedup 1.0000x reference)
"""Trainium2 Bass kernel for nn_MoELayer (B=4, L=2048, D=768, E=16, top-2, D_FF=3072).

Strategy (data-parallel over tokens): each of the 8 NeuronCores owns a
1024-token slice. Per core: fp32 router (logits -> softmax -> top-2 gating via
max8/match_replace), then for every expert a dense bf16 FFN over the slice,
gated accumulation into fp32, plus residual.

kernel(**inputs) takes the full unsharded numpy inputs and returns the full
[4, 2048, 768] fp32 output.
"""

import sys

sys.path.insert(0, "/opt/trn_rl_repo")

import numpy as np
import ml_dtypes

import concourse.bass as bass
import concourse.mybir as mybir
import concourse.tile as tile
from concourse import bacc
from concourse.bass_utils import run_bass_kernel_spmd

P = 128
D_MODEL = 768
D_FF = 3072
N_EXPERTS = 16
TOP_K = 2
B, L = 4, 2048
T_TOTAL = B * L  # 8192
N_CORES = 8
TC = T_TOTAL // N_CORES  # 1024 tokens per core
KD = D_MODEL // P  # 6 k-subtiles for d_model contraction
KF = D_FF // P  # 24 k-subtiles for d_ff contraction
KFA = KF + 1  # +1 augmented ones-row subtile (folds b2 into W2)
NT_TILES = TC // P  # 8 token tiles of 128
MM1_N = 512  # free-dim per matmul into one PSUM bank
FD = mybir.dt.float32
BF = mybir.dt.bfloat16
AF = mybir.ActivationFunctionType
AX = mybir.AxisListType


def build_router(tc, xTf, WrT, G_sb, rpool, psum_r):
    """fp32 router: logits = x @ Wr^T, softmax, top-2 gating (dense [128,tt,E])."""
    nc = tc.nc
    xTf_sb = rpool.tile([P, KD, TC], FD, tag="xTf")
    nc.sync.dma_start(xTf_sb[:], xTf[:].rearrange("(k p) t -> p k t", p=P))
    WrT_sb = rpool.tile([P, KD, N_EXPERTS], FD, tag="WrT")
    nc.sync.dma_start(WrT_sb[:], WrT[:].rearrange("(k p) e -> p k e", p=P))

    for tt in range(NT_TILES):
        ps = psum_r.tile([P, N_EXPERTS], FD, tag="ps_r")
        for k in range(KD):
            nc.tensor.matmul(
                ps[:],
                lhsT=xTf_sb[:, k, tt * P : (tt + 1) * P],
                rhs=WrT_sb[:, k, :],
                start=(k == 0),
                stop=(k == KD - 1),
            )
        nmax = rpool.tile([P, 1], FD, tag="nmax")
        nc.vector.reduce_max(nmax[:], ps[:], axis=AX.X, negate=True)
        ex = rpool.tile([P, N_EXPERTS], FD, tag="ex")
        ssum = rpool.tile([P, 1], FD, tag="ssum")
        nc.scalar.activation(ex[:], ps[:], AF.Exp, bias=nmax[:], accum_out=ssum[:])
        rs = rpool.tile([P, 1], FD, tag="rs")
        nc.vector.reciprocal(rs[:], ssum[:])
        mx8 = rpool.tile([P, 8], FD, tag="mx8")
        nc.vector.max(mx8[:], ex[:])
        nc.vector.memset(mx8[:, TOP_K:], 0.0)
        zap = rpool.tile([P, N_EXPERTS], FD, tag="zap")
        nc.vector.match_replace(
            zap[:], in_to_replace=mx8[:], in_values=ex[:], imm_value=0.0
        )
        gt = G_sb[:, tt, :]
        nc.vector.tensor_sub(gt, ex[:], zap[:])
        nc.vector.tensor_scalar_mul(gt, gt, rs[:])


def build_dense_moe(tc):
    """Emit the full per-core kernel IR. Returns nc."""
    nc = tc.nc
    # DRAM I/O (per-core shapes)
    xTf = nc.dram_tensor("xTf", [D_MODEL, TC], FD, kind="ExternalInput")
    xTbf = nc.dram_tensor("xTbf", [D_MODEL, TC], BF, kind="ExternalInput")
    xs = nc.dram_tensor("xs", [TC, D_MODEL], FD, kind="ExternalInput")
    WrT = nc.dram_tensor("WrT", [D_MODEL, N_EXPERTS], FD, kind="ExternalInput")
    W1bf = nc.dram_tensor("W1bf", [N_EXPERTS, D_MODEL, D_FF], BF, kind="ExternalInput")
    b1 = nc.dram_tensor("b1", [N_EXPERTS, D_FF], FD, kind="ExternalInput")
    W2a = nc.dram_tensor("W2a", [N_EXPERTS, KFA * P, D_MODEL], BF, kind="ExternalInput")
    y_out = nc.dram_tensor("y", [TC, D_MODEL], FD, kind="ExternalOutput")

    import contextlib

    with contextlib.ExitStack() as ctx:
        cpool = ctx.enter_context(tc.tile_pool(name="const", bufs=1))
        G_sb = cpool.tile([P, NT_TILES, N_EXPERTS], FD)  # gating, per token tile
        y_acc = cpool.tile([P, NT_TILES, D_MODEL], FD)  # accumulated expert outputs
        hones = cpool.tile([P, TC], BF)  # ones row for b2 folding
        nc.vector.memset(hones[:], 0.0)
        nc.vector.memset(hones[0:1, :], 1.0)
        nc.vector.memset(y_acc[:], 0.0)

        # ---- Router phase (own pools; SBUF released before FFN) ----
        with tc.tile_pool(name="router", bufs=2) as rpool, tc.tile_pool(
            name="psum_r", bufs=1, space="PSUM"
        ) as psum_r:
            build_router(tc, xTf, WrT, G_sb, rpool, psum_r)

        # ---- FFN phase ----
        xpool = ctx.enter_context(tc.tile_pool(name="xbf", bufs=1))
        xTbf_sb = xpool.tile([P, KD, TC], BF)
        nc.sync.dma_start(xTbf_sb[:], xTbf[:].rearrange("(k p) t -> p k t", p=P))

        w1pool = ctx.enter_context(tc.tile_pool(name="w1", bufs=KD + 1))
        w2pool = ctx.enter_context(tc.tile_pool(name="w2", bufs=KFA + 1))
        bpool = ctx.enter_context(tc.tile_pool(name="b1", bufs=2))
        hpool = ctx.enter_context(tc.tile_pool(name="hT", bufs=1))
        tpool = ctx.enter_context(tc.tile_pool(name="tmp", bufs=3))
        psum1 = ctx.enter_context(tc.tile_pool(name="psum1", bufs=2, space="PSUM"))
        psum2a = ctx.enter_context(tc.tile_pool(name="psum2a", bufs=2, space="PSUM"))
        psum2b = ctx.enter_context(tc.tile_pool(name="psum2b", bufs=2, space="PSUM"))

        for e in range(N_EXPERTS):
            # mm1: hT[f, t] = gelu(sum_d W1[d, f] * xT[d, t] + b1[f])
            w1t = []
            for k in range(KD):
                w = w1pool.tile([P, D_FF], BF, tag="w1k")
                nc.sync.dma_start(w[:], W1bf[e, k * P : (k + 1) * P, :])
                w1t.append(w)
            b1t = bpool.tile([P, KF], FD, tag="b1t")
            nc.sync.dma_start(b1t[:], b1[e].rearrange("(o p) -> p o", p=P))
            hT = hpool.tile([P, KF, TC], BF, tag="hT")
            for mt in range(KF):
                for nt in range(TC // MM1_N):
                    ps = psum1.tile([P, MM1_N], FD, tag="ps1")
                    for k in range(KD):
                        nc.tensor.matmul(
                            ps[:],
                            lhsT=w1t[k][:, mt * P : (mt + 1) * P],
                            rhs=xTbf_sb[:, k, nt * MM1_N : (nt + 1) * MM1_N],
                            start=(k == 0),
                            stop=(k == KD - 1),
                        )
                    nc.scalar.activation(
                        hT[:, mt, nt * MM1_N : (nt + 1) * MM1_N],
                        ps[:],
                        AF.Gelu,
                        bias=b1t[:, mt : mt + 1],
                    )
            # mm2: out[t, d] = sum_f hT[f, t] * W2a[f, d]  (+b2 via ones row)
            w2t = []
            for k in range(KFA):
                w = w2pool.tile([P, D_MODEL], BF, tag="w2k")
                nc.sync.dma_start(w[:], W2a[e, k * P : (k + 1) * P, :])
                w2t.append(w)
            for tt in range(NT_TILES):
                psa = psum2a.tile([P, MM1_N], FD, tag="ps2a")
                psb = psum2b.tile([P, D_MODEL - MM1_N], FD, tag="ps2b")
                for k in range(KFA):
                    lhs = (
                        hT[:, k, tt * P : (tt + 1) * P]
                        if k < KF
                        else hones[:, tt * P : (tt + 1) * P]
                    )
                    nc.tensor.matmul(
                        psa[:], lhsT=lhs, rhs=w2t[k][:, :MM1_N],
                        start=(k == 0), stop=(k == KFA - 1),
                    )
                    nc.tensor.matmul(
                        psb[:], lhsT=lhs, rhs=w2t[k][:, MM1_N:],
                        start=(k == 0), stop=(k == KFA - 1),
                    )
                g_ap = G_sb[:, tt, e : e + 1]
                tmp = tpool.tile([P, D_MODEL], FD, tag="tmp")
                nc.scalar.activation(tmp[:, :MM1_N], psa[:], AF.Copy, scale=g_ap)
                nc.scalar.activation(tmp[:, MM1_N:], psb[:], AF.Copy, scale=g_ap)
                nc.vector.tensor_add(y_acc[:, tt, :], y_acc[:, tt, :], tmp[:])

        # ---- residual + writeback ----
        opool = ctx.enter_context(tc.tile_pool(name="outp", bufs=3))
        for tt in range(NT_TILES):
            xt = opool.tile([P, D_MODEL], FD, tag="xt")
            nc.sync.dma_start(xt[:], xs[tt * P : (tt + 1) * P, :])
            nc.vector.tensor_add(xt[:], xt[:], y_acc[:, tt, :])
            nc.sync.dma_start(y_out[tt * P : (tt + 1) * P, :], xt[:])
    return nc


_NC_CACHE = None


def get_nc():
    global _NC_CACHE
    if _NC_CACHE is None:
        nc = bacc.Bacc(None, target_bir_lowering=False)
        with tile.TileContext(nc) as tcx:
            build_dense_moe(tcx)
        nc.compile()
        _NC_CACHE = nc
    return _NC_CACHE


def prepare_in_maps(x, Wr, W1, b1, W2, b2):
    x2 = np.ascontiguousarray(x.reshape(T_TOTAL, D_MODEL).astype(np.float32))
    xT = np.ascontiguousarray(x2.T)
    xTbf = xT.astype(ml_dtypes.bfloat16)
    WrT = np.ascontiguousarray(Wr.astype(np.float32).T)
    W1bf = W1.astype(ml_dtypes.bfloat16)
    W2a = np.concatenate(
        [
            W2.astype(np.float32),
            b2.astype(np.float32)[:, None, :],
            np.zeros((N_EXPERTS, P - 1, D_MODEL), np.float32),
        ],
        axis=1,
    ).astype(ml_dtypes.bfloat16)
    b1f = np.ascontiguousarray(b1.astype(np.float32))

    in_maps = []
    for c in range(N_CORES):
        sl = slice(c * TC, (c + 1) * TC)
        in_maps.append(
            {
                "xTf": np.ascontiguousarray(xT[:, sl]),
                "xTbf": np.ascontiguousarray(xTbf[:, sl]),
                "xs": np.ascontiguousarray(x2[sl]),
                "WrT": WrT,
                "W1bf": W1bf,
                "b1": b1f,
                "W2a": W2a,
            }
        )
    return in_maps


def kernel(x, Wr, W1, b1, W2, b2, _trace=False):
    nc = get_nc()
    in_maps = prepare_in_maps(x, Wr, W1, b1, W2, b2)
    res = run_bass_kernel_spmd(
        nc, in_maps, core_ids=list(range(N_CORES)), trace=_trace
    )
    out = np.concatenate([res.results[c]["y"] for c in range(N_CORES)], axis=0)
    out = out.reshape(B, L, D_MODEL).astype(x.dtype)
    if _trace:
        kernel.last_result = res
    return out


# revision 4
# speedup vs baseline: 2.5979x; 2.5979x over previous
"""Trainium2 Bass kernel for nn_MoELayer (B=4, L=2048, D=768, E=16, top-2, D_FF=3072).

Strategy (data-parallel over tokens): each of the 8 NeuronCores owns a
1024-token slice. Per core: fp32 router (logits -> softmax -> top-2 gating via
max8/match_replace), then for every expert a dense bf16 FFN over the slice,
gated accumulation into fp32, plus residual.

kernel(**inputs) takes the full unsharded numpy inputs and returns the full
[4, 2048, 768] fp32 output.
"""

import sys

sys.path.insert(0, "/opt/trn_rl_repo")

import numpy as np
import ml_dtypes

import concourse.bass as bass
import concourse.mybir as mybir
import concourse.tile as tile
from concourse import bacc
from concourse.bass_utils import run_bass_kernel_spmd

P = 128
D_MODEL = 768
D_FF = 3072
N_EXPERTS = 16
TOP_K = 2
B, L = 4, 2048
T_TOTAL = B * L  # 8192
N_CORES = 8
TC = T_TOTAL // N_CORES  # 1024 tokens per core
KD = D_MODEL // P  # 6 k-subtiles for d_model contraction
KF = D_FF // P  # 24 k-subtiles for d_ff contraction
KFA = KF + 1  # +1 augmented ones-row subtile (folds b2 into W2)
NT_TILES = TC // P  # 8 token tiles of 128
MM1_N = 512  # free-dim per matmul into one PSUM bank
FD = mybir.dt.float32
BF = mybir.dt.bfloat16
AF = mybir.ActivationFunctionType
AX = mybir.AxisListType


def build_router(tc, xTf, WrT, G_sb, rpool, psum_r):
    """fp32 router: logits = x @ Wr^T, softmax, top-2 gating (dense [128,tt,E])."""
    nc = tc.nc
    xTf_sb = rpool.tile([P, KD, TC], FD, tag="xTf")
    nc.sync.dma_start(xTf_sb[:], xTf[:].rearrange("(k p) t -> p k t", p=P))
    WrT_sb = rpool.tile([P, KD, N_EXPERTS], FD, tag="WrT")
    nc.sync.dma_start(WrT_sb[:], WrT[:].rearrange("(k p) e -> p k e", p=P))

    for tt in range(NT_TILES):
        ps = psum_r.tile([P, N_EXPERTS], FD, tag="ps_r")
        for k in range(KD):
            nc.tensor.matmul(
                ps[:],
                lhsT=xTf_sb[:, k, tt * P : (tt + 1) * P],
                rhs=WrT_sb[:, k, :],
                start=(k == 0),
                stop=(k == KD - 1),
            )
        nmax = rpool.tile([P, 1], FD, tag="nmax")
        nc.vector.reduce_max(nmax[:], ps[:], axis=AX.X, negate=True)
        ex = rpool.tile([P, N_EXPERTS], FD, tag="ex")
        ssum = rpool.tile([P, 1], FD, tag="ssum")
        nc.scalar.activation(ex[:], ps[:], AF.Exp, bias=nmax[:], accum_out=ssum[:])
        rs = rpool.tile([P, 1], FD, tag="rs")
        nc.vector.reciprocal(rs[:], ssum[:])
        mx8 = rpool.tile([P, 8], FD, tag="mx8")
        nc.vector.max(mx8[:], ex[:])
        nc.vector.memset(mx8[:, TOP_K:], 0.0)
        zap = rpool.tile([P, N_EXPERTS], FD, tag="zap")
        nc.vector.match_replace(
            zap[:], in_to_replace=mx8[:], in_values=ex[:], imm_value=0.0
        )
        gt = G_sb[:, tt, :]
        nc.vector.tensor_sub(gt, ex[:], zap[:])
        nc.vector.tensor_scalar_mul(gt, gt, rs[:])


def build_dense_moe(tc):
    """Emit the full per-core kernel IR. Returns nc."""
    nc = tc.nc
    # DRAM I/O (per-core shapes)
    xTf = nc.dram_tensor("xTf", [D_MODEL, TC], FD, kind="ExternalInput")
    xTbf = nc.dram_tensor("xTbf", [D_MODEL, TC], BF, kind="ExternalInput")
    xs = nc.dram_tensor("xs", [TC, D_MODEL], FD, kind="ExternalInput")
    WrT = nc.dram_tensor("WrT", [D_MODEL, N_EXPERTS], FD, kind="ExternalInput")
    W1bf = nc.dram_tensor("W1bf", [N_EXPERTS, D_MODEL, D_FF], BF, kind="ExternalInput")
    b1 = nc.dram_tensor("b1", [N_EXPERTS, D_FF], FD, kind="ExternalInput")
    W2a = nc.dram_tensor("W2a", [N_EXPERTS, KFA * P, D_MODEL], BF, kind="ExternalInput")
    y_out = nc.dram_tensor("y", [TC, D_MODEL], FD, kind="ExternalOutput")

    import contextlib

    with contextlib.ExitStack() as ctx:
        cpool = ctx.enter_context(tc.tile_pool(name="const", bufs=1))
        G_sb = cpool.tile([P, NT_TILES, N_EXPERTS], FD)  # gating, per token tile
        y_acc = cpool.tile([P, NT_TILES, D_MODEL], FD)  # accumulated expert outputs
        hones = cpool.tile([P, TC], BF)  # ones row for b2 folding
        nc.vector.memset(hones[:], 0.0)
        nc.vector.memset(hones[0:1, :], 1.0)
        nc.vector.memset(y_acc[:], 0.0)

        # ---- Router phase (own pools; SBUF released before FFN) ----
        with tc.tile_pool(name="router", bufs=2) as rpool, tc.tile_pool(
            name="psum_r", bufs=1, space="PSUM"
        ) as psum_r:
            build_router(tc, xTf, WrT, G_sb, rpool, psum_r)

        # ---- FFN phase ----
        xpool = ctx.enter_context(tc.tile_pool(name="xbf", bufs=1))
        xTbf_sb = xpool.tile([P, KD, TC], BF)
        nc.sync.dma_start(xTbf_sb[:], xTbf[:].rearrange("(k p) t -> p k t", p=P))

        w1pool = ctx.enter_context(tc.tile_pool(name="w1", bufs=KD + 1))
        w2pool = ctx.enter_context(tc.tile_pool(name="w2", bufs=KFA + 1))
        bpool = ctx.enter_context(tc.tile_pool(name="b1", bufs=2))
        hpool = ctx.enter_context(tc.tile_pool(name="hT", bufs=1))
        tpool = ctx.enter_context(tc.tile_pool(name="tmp", bufs=3))
        psum1 = ctx.enter_context(tc.tile_pool(name="psum1", bufs=2, space="PSUM"))
        psum2a = ctx.enter_context(tc.tile_pool(name="psum2a", bufs=2, space="PSUM"))
        psum2b = ctx.enter_context(tc.tile_pool(name="psum2b", bufs=2, space="PSUM"))

        for e in range(N_EXPERTS):
            # mm1: hT[f, t] = gelu(sum_d W1[d, f] * xT[d, t] + b1[f])
            w1t = []
            for k in range(KD):
                w = w1pool.tile([P, D_FF], BF, tag="w1k")
                nc.sync.dma_start(w[:], W1bf[e, k * P : (k + 1) * P, :])
                w1t.append(w)
            b1t = bpool.tile([P, KF], FD, tag="b1t")
            nc.sync.dma_start(b1t[:], b1[e].rearrange("(o p) -> p o", p=P))
            hT = hpool.tile([P, KF, TC], BF, tag="hT")
            for mt in range(KF):
                for nt in range(TC // MM1_N):
                    ps = psum1.tile([P, MM1_N], FD, tag="ps1")
                    for k in range(KD):
                        nc.tensor.matmul(
                            ps[:],
                            lhsT=w1t[k][:, mt * P : (mt + 1) * P],
                            rhs=xTbf_sb[:, k, nt * MM1_N : (nt + 1) * MM1_N],
                            start=(k == 0),
                            stop=(k == KD - 1),
                        )
                    nc.scalar.activation(
                        hT[:, mt, nt * MM1_N : (nt + 1) * MM1_N],
                        ps[:],
                        AF.Gelu,
                        bias=b1t[:, mt : mt + 1],
                    )
            # mm2: out[t, d] = sum_f hT[f, t] * W2a[f, d]  (+b2 via ones row)
            w2t = []
            for k in range(KFA):
                w = w2pool.tile([P, D_MODEL], BF, tag="w2k")
                nc.sync.dma_start(w[:], W2a[e, k * P : (k + 1) * P, :])
                w2t.append(w)
            for tt in range(NT_TILES):
                psa = psum2a.tile([P, MM1_N], FD, tag="ps2a")
                psb = psum2b.tile([P, D_MODEL - MM1_N], FD, tag="ps2b")
                for k in range(KFA):
                    lhs = (
                        hT[:, k, tt * P : (tt + 1) * P]
                        if k < KF
                        else hones[:, tt * P : (tt + 1) * P]
                    )
                    nc.tensor.matmul(
                        psa[:], lhsT=lhs, rhs=w2t[k][:, :MM1_N],
                        start=(k == 0), stop=(k == KFA - 1),
                    )
                    nc.tensor.matmul(
                        psb[:], lhsT=lhs, rhs=w2t[k][:, MM1_N:],
                        start=(k == 0), stop=(k == KFA - 1),
                    )
                g_ap = G_sb[:, tt, e : e + 1]
                tmp = tpool.tile([P, D_MODEL], FD, tag="tmp")
                nc.scalar.activation(tmp[:, :MM1_N], psa[:], AF.Copy, scale=g_ap)
                nc.scalar.activation(tmp[:, MM1_N:], psb[:], AF.Copy, scale=g_ap)
                nc.vector.tensor_add(y_acc[:, tt, :], y_acc[:, tt, :], tmp[:])

        # ---- residual + writeback ----
        opool = ctx.enter_context(tc.tile_pool(name="outp", bufs=3))
        for tt in range(NT_TILES):
            xt = opool.tile([P, D_MODEL], FD, tag="xt")
            nc.sync.dma_start(xt[:], xs[tt * P : (tt + 1) * P, :])
            nc.vector.tensor_add(xt[:], xt[:], y_acc[:, tt, :])
            nc.sync.dma_start(y_out[tt * P : (tt + 1) * P, :], xt[:])
    return nc


_NC_CACHE = {}


def get_nc(mode="dense"):
    if mode not in _NC_CACHE:
        if mode == "dense":
            nc = bacc.Bacc(None, target_bir_lowering=False)
            with tile.TileContext(nc) as tcx:
                build_dense_moe(tcx)
        else:
            import sparse_kernel as S

            nc = bacc.Bacc(None, target_bir_lowering=False, num_devices=N_CORES)
            with tile.TileContext(nc) as tcx:
                S.build_sparse_core(
                    tcx, T_TOTAL, CAP, n_cores=N_CORES, collective=True, half_tiles=5
                )
        nc.compile()
        _NC_CACHE[mode] = nc
    return _NC_CACHE[mode]


CAP = 1280  # capacity slots per expert (mean load 1024)


def prepare_in_maps(x, Wr, W1, b1, W2, b2):
    x2 = np.ascontiguousarray(x.reshape(T_TOTAL, D_MODEL).astype(np.float32))
    xT = np.ascontiguousarray(x2.T)
    xTbf = xT.astype(ml_dtypes.bfloat16)
    WrT = np.ascontiguousarray(Wr.astype(np.float32).T)
    W1bf = W1.astype(ml_dtypes.bfloat16)
    W2a = np.concatenate(
        [
            W2.astype(np.float32),
            b2.astype(np.float32)[:, None, :],
            np.zeros((N_EXPERTS, P - 1, D_MODEL), np.float32),
        ],
        axis=1,
    ).astype(ml_dtypes.bfloat16)
    b1f = np.ascontiguousarray(b1.astype(np.float32))

    in_maps = []
    for c in range(N_CORES):
        sl = slice(c * TC, (c + 1) * TC)
        in_maps.append(
            {
                "xTf": np.ascontiguousarray(xT[:, sl]),
                "xTbf": np.ascontiguousarray(xTbf[:, sl]),
                "xs": np.ascontiguousarray(x2[sl]),
                "WrT": WrT,
                "W1bf": W1bf,
                "b1": b1f,
                "W2a": W2a,
            }
        )
    return in_maps


def kernel_dense(x, Wr, W1, b1, W2, b2, _trace=False, **trace_kw):
    nc = get_nc("dense")
    in_maps = prepare_in_maps(x, Wr, W1, b1, W2, b2)
    res = run_bass_kernel_spmd(
        nc, in_maps, core_ids=list(range(N_CORES)), trace=_trace, **trace_kw
    )
    out = np.concatenate([res.results[c]["y"] for c in range(N_CORES)], axis=0)
    out = out.reshape(B, L, D_MODEL).astype(x.dtype)
    if _trace:
        kernel.last_result = res
    return out


def kernel(x, Wr, W1, b1, W2, b2, _trace=False, **trace_kw):
    import sparse_kernel as S

    nc = get_nc("sparse")
    x2 = np.ascontiguousarray(x.reshape(T_TOTAL, D_MODEL).astype(np.float32))
    in_maps = [
        S.host_inputs_for_core(c, T_TOTAL, x2, Wr, W1, b1, W2, b2, n_cores=N_CORES)
        for c in range(N_CORES)
    ]
    res = run_bass_kernel_spmd(
        nc, in_maps, core_ids=list(range(N_CORES)), trace=_trace, **trace_kw
    )
    y_ig = np.concatenate([res.results[c]["y"] for c in range(N_CORES)], axis=0)
    sig = S.sigma_perm(T_TOTAL)
    out = y_ig[sig].reshape(B, L, D_MODEL).astype(x.dtype)
    if _trace:
        kernel.last_result = res
    return out
